# revision 5
# baseline (speedup 1.0000x reference)
"""Trainium2 Bass kernel for CatFeaturesItemNet (EmbeddingBag sum, segment_reduce).

Data-parallel over items, embedding table replicated (per sharding hint):
8 cores x 8192 items. Host prep builds per-core gather index + mask streams;
device does all payload movement and reduction.

v3 design:
  * Table uploaded in bf16, viewed as [25000, 512] (4 rows per 1KB block) so
    block ids fit dma_gather's int16 index range.  SWDGE gather is
    descriptor-rate-bound (~5ns/desc on 4 queues) up to 1KB elems, so bf16
    halves gathered bytes for free.
  * Gathers split round-robin across all 4 SWDGE queues (num_swdge_queues=4).
  * Items sorted by bag length; static binomial lane schedule gives each
    group of 128 items a fixed lane count L.  Runs of equal-L groups are
    packed into supergroup tiles (G*L <= 32 blocks/partition).
  * DVE per supergroup: ONE in-place contiguous bf16 multiply by the
    phase-select/pad mask (broadcast over d), then a contiguous in-place
    halving tree over the (lane, phase) axis; the final fold writes f32.
    No strided reduce (measured 9.5us/group vs ~0.3us/op for contiguous TT).
  * Host unpermutes rows back to original item order.
"""

import numpy as np
from contextlib import ExitStack

N_CORES = 8
BATCH = 65536
BL = BATCH // N_CORES          # items per core
L_MAX = 16
D = 128
V = 100000                     # weight rows
NBLK = V // 4                  # 1KB bf16 blocks (4 rows each)
EBF = 512                      # bf16 elems per gathered block (1KB)
GROUPS = BL // 128             # 64 groups of 128 items per core
SAFETY_SIGMA = 10.0
NQUEUES = 4
SG_BUDGET = 32                 # max G*L blocks per supergroup tile


def _static_lane_schedule(n_items=BL, groups=GROUPS):
    """L_hat[g]: static upper bound on the max bag length within group g of
    128 items after sorting lengths (uniform{1..16}) in descending order.
    Group g's max length exceeds L only if count(len >= L+1) > 128*g;
    count(len >= k) ~ Binomial(n, (17-k)/16)."""
    sched = []
    for g in range(groups):
        lhat = L_MAX
        for L in range(1, L_MAX + 1):
            p = (L_MAX - L) / 16.0  # P(len >= L+1) for len ~ uniform{1..16}
            mean = n_items * p
            sigma = np.sqrt(n_items * p * (1 - p))
            if mean + SAFETY_SIGMA * sigma <= g * 128:
                lhat = L
                break
        sched.append(lhat)
    return sched


L_SCHED = _static_lane_schedule()
ICOLS = sum(128 * L // 16 for L in L_SCHED)        # int16 idx cols
MCOLS = sum(L * 4 for L in L_SCHED)                # bf16 mask cols


def _supergroups():
    """Pack runs of equal-L groups into supergroups of G groups each,
    G*L <= SG_BUDGET.  Returns list of (g0, G, L)."""
    sgs = []
    g = 0
    while g < GROUPS:
        L = L_SCHED[g]
        run = 1
        while g + run < GROUPS and L_SCHED[g + run] == L:
            run += 1
        cap = max(1, SG_BUDGET // L)
        while run > 0:
            take = min(run, cap)
            sgs.append((g, take, L))
            g += take
            run -= take
    return sgs


SGS = _supergroups()


def build_bass(reps=1, stages=("gather", "mult", "fold", "store")):
    """Compile the 8-core SPMD program. reps>1 loops the whole pipeline
    on-device (used by test.py for delta timing)."""
    import concourse.bacc as bacc
    import concourse.tile as tile
    from concourse import mybir
    from concourse.library_config import mlp

    nc = bacc.Bacc("TRN2", target_bir_lowering=False, debug=False,
                   num_devices=N_CORES, num_swdge_queues=NQUEUES)
    weight = nc.declare_dram_parameter("weight", [NBLK, EBF],
                                       mybir.dt.bfloat16, isOutput=False)
    idx_in = nc.declare_dram_parameter("idx", [128, ICOLS], mybir.dt.int16,
                                       isOutput=False)
    mask_in = nc.declare_dram_parameter("mask", [128, MCOLS],
                                        mybir.dt.bfloat16, isOutput=False)
    out = nc.declare_dram_parameter("out", [BL, D], mybir.dt.float32,
                                    isOutput=True)
    # row g*128+p -> [p][g][d]
    out_p = out.rearrange("(g p) d -> p g d", p=128)

    qn = 0
    with tile.TileContext(nc) as tc:
        with ExitStack() as ctx:
            cons = ctx.enter_context(tc.tile_pool(name="cons", bufs=1))
            gp = ctx.enter_context(tc.tile_pool(name="g", bufs=4))
            op = ctx.enter_context(tc.tile_pool(name="o", bufs=4))

            nc.gpsimd.load_library(mlp)
            idx_t = cons.tile([128, ICOLS], mybir.dt.int16)
            nc.sync.dma_start(out=idx_t[:], in_=idx_in[:, :])
            mask_t = cons.tile([128, MCOLS], mybir.dt.bfloat16)
            nc.sync.dma_start(out=mask_t[:], in_=mask_in[:, :])

            with tc.For_i(0, reps) as _i:
                # Software-pipelined emission: W supergroups' op chains are
                # interleaved instruction-by-instruction so the in-order DVE
                # queue always has a ready op while a chain waits on its
                # predecessor's semaphore (chained DVE ops otherwise cost
                # ~2.4us each in exposed latency).
                W = 4

                def sg_ops(g0, G, L, ic, mc, qbase):
                    T4 = L * 4
                    ni = 128 * L
                    GT4 = G * T4
                    gt = gp.tile([128, G * L * EBF], mybir.dt.bfloat16,
                                 tag="g", name=f"gt{g0}")
                    ot = op.tile([128, G * D], mybir.dt.float32, tag="o",
                                 name=f"ot{g0}")
                    if "gather" in stages:
                        for k in range(G):
                            def do_gather(k=k, gt=gt, ic=ic, ni=ni, L=L,
                                          q=(qbase + k) % NQUEUES):
                                sub = gt[:, k * L * EBF:(k + 1) * L * EBF]
                                nc.gpsimd.dma_gather(
                                    out_ap=sub.rearrange(
                                        "p (c e) -> p c e", e=EBF),
                                    in_ap=weight[:, :],
                                    idxs_ap=idx_t[:, ic + k * ni // 16:
                                                  ic + (k + 1) * ni // 16],
                                    num_idxs=ni,
                                    num_idxs_reg=ni,
                                    elem_size=EBF,
                                    single_packet=False,
                                    queue_num=q,
                                )
                            yield do_gather
                    if "mult" in stages:
                        def do_mult(gt=gt, GT4=GT4, mc=mc):
                            g3 = gt[:].rearrange("p (t d) -> p t d", d=D)
                            m3 = mask_t[:, mc:mc + GT4].to_broadcast(
                                [128, GT4, D])
                            nc.vector.tensor_tensor(
                                out=g3, in0=g3, in1=m3,
                                op=mybir.AluOpType.mult)
                        yield do_mult
                    if "fold" in stages:
                        t = T4
                        while t > 2:
                            h = t // 2
                            def do_fold(gt=gt, G=G, T4=T4, t=t, h=h):
                                g4 = gt[:].rearrange(
                                    "p (g t d) -> p g t d", g=G, t=T4)
                                nc.vector.tensor_tensor(
                                    out=g4[:, :, 0:h, :],
                                    in0=g4[:, :, 0:h, :],
                                    in1=g4[:, :, t - h:t, :],
                                    op=mybir.AluOpType.add)
                            yield do_fold
                            t = t - h

                        def do_final(gt=gt, ot=ot, G=G, T4=T4):
                            g4 = gt[:].rearrange(
                                "p (g t d) -> p g t d", g=G, t=T4)
                            o3 = ot[:].rearrange("p (g d) -> p g d", d=D)
                            nc.vector.tensor_tensor(
                                out=o3, in0=g4[:, :, 0, :],
                                in1=g4[:, :, 1, :],
                                op=mybir.AluOpType.add)
                        yield do_final
                    if "store" in stages:
                        def do_store(ot=ot, g0=g0, G=G):
                            nc.sync.dma_start(
                                out=out_p[:, g0:g0 + G, :],
                                in_=ot[:].rearrange("p (g d) -> p g d", d=D))
                        yield do_store

                gens = []
                ic = 0
                mc = 0
                qn = 0
                for (g0, G, L) in SGS:
                    gens.append(sg_ops(g0, G, L, ic, mc, qn))
                    ic += G * (128 * L) // 16
                    mc += G * L * 4
                    qn += G
                from collections import deque
                window = deque()
                rest = deque(gens)
                while rest or window:
                    while len(window) < W and rest:
                        window.append(rest.popleft())
                    gen = window.popleft()
                    try:
                        step = next(gen)
                    except StopIteration:
                        continue
                    step()
                    window.append(gen)
    nc.compile()
    return nc


def _host_prep(core_items, emb_bag_inputs, offsets, input_lengths):
    """Build per-core idx/mask tensors + the inverse permutation."""
    import ml_dtypes
    it = core_items.astype(np.int64)
    off = offsets[it].astype(np.int64)
    ln = input_lengths[it].astype(np.int64)
    ids = emb_bag_inputs[off[:, None] + np.arange(L_MAX)[None, :]].astype(np.int64)

    order = np.argsort(-ln, kind="stable")      # items sorted by len desc
    ln_s = ln[order]
    ids_s = ids[order]

    idx_arr = np.zeros((128, ICOLS), dtype=np.int16)
    mask_arr = np.zeros((128, MCOLS), dtype=ml_dtypes.bfloat16)
    ic = 0
    mc = 0
    for g, L in enumerate(L_SCHED):
        sl = slice(g * 128, (g + 1) * 128)
        ln_g = ln_s[sl]                          # [128]
        if ln_g.max(initial=0) > L:
            raise RuntimeError(
                f"static lane schedule violated in group {g}: "
                f"max len {ln_g.max()} > {L}")
        ids_g = ids_s[sl]                        # [128, 16]
        lanes = np.minimum(np.arange(L)[None, :], ln_g[:, None] - 1)
        lane_ids = np.take_along_axis(ids_g, lanes, axis=1)  # [128, L]
        blk = (lane_ids >> 2).astype(np.int16)               # [128, L]
        ph = (lane_ids & 3).astype(np.int64)                 # [128, L]
        valid = (np.arange(L)[None, :] < ln_g[:, None])      # [128, L]

        # column-major stream: s = t*128 + p
        stream = blk.T.reshape(-1)                           # [128*L]
        ni = 128 * L
        wrapped = stream.reshape(ni // 16, 16).T             # [16, ni/16]
        idx_arr[:, ic:ic + ni // 16] = np.tile(wrapped, (8, 1))

        m = np.zeros((128, L, 4), dtype=np.float32)
        np.put_along_axis(m, ph[:, :, None], 1.0, axis=2)
        m *= valid[:, :, None]
        mask_arr[:, mc:mc + L * 4] = m.reshape(128, L * 4).astype(
            ml_dtypes.bfloat16)
        ic += ni // 16
        mc += L * 4

    inv = np.empty(BL, dtype=np.int64)
    inv[order] = np.arange(BL)                  # original j -> sorted row
    return idx_arr, mask_arr, inv


def _host_fallback(items, emb_bag_inputs, offsets, input_lengths, weight):
    """Exact numpy fallback (used only if the static schedule is violated,
    i.e. the input distribution differs from the spec)."""
    it = items.astype(np.int64)
    off = offsets[it].astype(np.int64)
    ln = input_lengths[it].astype(np.int64)
    r = np.arange(L_MAX)
    idx = off[:, None] + r[None, :]
    msk = r[None, :] < ln[:, None]
    fid = emb_bag_inputs[np.where(msk, idx, 0)]
    return (weight[fid] * msk[:, :, None].astype(np.float32)).sum(axis=1)


_CACHE = {}


def kernel(items, emb_bag_inputs, offsets, input_lengths, weight):
    import ml_dtypes
    from concourse.bass_utils import run_bass_kernel_spmd

    items = np.asarray(items)
    emb_bag_inputs = np.asarray(emb_bag_inputs)
    offsets = np.asarray(offsets)
    input_lengths = np.asarray(input_lengths)
    weight = np.asarray(weight)

    if "nc" not in _CACHE:
        _CACHE["nc"] = build_bass()
    nc = _CACHE["nc"]

    wblk = np.ascontiguousarray(weight, dtype=np.float32).astype(
        ml_dtypes.bfloat16).reshape(NBLK, EBF)
    in_maps = []
    invs = []
    for c in range(N_CORES):
        try:
            idx_arr, mask_arr, inv = _host_prep(
                items[c * BL:(c + 1) * BL], emb_bag_inputs, offsets,
                input_lengths)
        except RuntimeError:
            return _host_fallback(items, emb_bag_inputs, offsets,
                                  input_lengths,
                                  weight.astype(np.float32)).astype(np.float32)
        in_maps.append({"weight": wblk, "idx": idx_arr, "mask": mask_arr})
        invs.append(inv)

    res = run_bass_kernel_spmd(nc, in_maps, list(range(N_CORES)))
    outs = []
    for c in range(N_CORES):
        dev = res.results[c]["out"]            # [BL, D] in sorted order
        outs.append(dev[invs[c]])
    return np.concatenate(outs, axis=0).astype(np.float32)


# revision 6
# speedup vs baseline: 1.3336x; 1.3336x over previous
"""Trainium2 Bass kernel for CatFeaturesItemNet (EmbeddingBag sum, segment_reduce).

Data-parallel over items, embedding table replicated (per sharding hint):
8 cores x 8192 items. Host prep builds per-core gather index + mask streams;
device does all payload movement and reduction.

v3 design:
  * Table uploaded in bf16, viewed as [25000, 512] (4 rows per 1KB block) so
    block ids fit dma_gather's int16 index range.  SWDGE gather is
    descriptor-rate-bound (~5ns/desc on 4 queues) up to 1KB elems, so bf16
    halves gathered bytes for free.
  * Gathers split round-robin across all 4 SWDGE queues (num_swdge_queues=4).
  * Items sorted by bag length; static binomial lane schedule gives each
    group of 128 items a fixed lane count L.  Runs of equal-L groups are
    packed into supergroup tiles (G*L <= 32 blocks/partition).
  * DVE per supergroup: ONE in-place contiguous bf16 multiply by the
    phase-select/pad mask (broadcast over d), then a contiguous in-place
    halving tree over the (lane, phase) axis; the final fold writes f32.
    No strided reduce (measured 9.5us/group vs ~0.3us/op for contiguous TT).
  * Host unpermutes rows back to original item order.
"""

import numpy as np
from contextlib import ExitStack

N_CORES = 8
BATCH = 65536
BL = BATCH // N_CORES          # items per core
L_MAX = 16
D = 128
V = 100000                     # weight rows
NBLK = V // 4                  # 1KB bf16 blocks (4 rows each)
EBF = 512                      # bf16 elems per gathered block (1KB)
GROUPS = BL // 128             # 64 groups of 128 items per core
SAFETY_SIGMA = 10.0
NQUEUES = 4
SG_BUDGET = 32                 # max G*L blocks per supergroup tile


def _static_lane_schedule(n_items=BL, groups=GROUPS):
    """L_hat[g]: static upper bound on the max bag length within group g of
    128 items after sorting lengths (uniform{1..16}) in descending order.
    Group g's max length exceeds L only if count(len >= L+1) > 128*g;
    count(len >= k) ~ Binomial(n, (17-k)/16)."""
    sched = []
    for g in range(groups):
        lhat = L_MAX
        for L in range(1, L_MAX + 1):
            p = (L_MAX - L) / 16.0  # P(len >= L+1) for len ~ uniform{1..16}
            mean = n_items * p
            sigma = np.sqrt(n_items * p * (1 - p))
            if mean + SAFETY_SIGMA * sigma <= g * 128:
                lhat = L
                break
        sched.append(lhat)
    return sched


L_SCHED = _static_lane_schedule()
ICOLS = sum(128 * L // 16 for L in L_SCHED)        # int16 idx cols
MCOLS = sum(L * 4 for L in L_SCHED)                # bf16 mask cols


def _supergroups():
    """Pack runs of equal-L groups into supergroups of G groups each,
    G*L <= SG_BUDGET.  Returns list of (g0, G, L)."""
    sgs = []
    g = 0
    while g < GROUPS:
        L = L_SCHED[g]
        run = 1
        while g + run < GROUPS and L_SCHED[g + run] == L:
            run += 1
        cap = max(1, SG_BUDGET // L)
        while run > 0:
            take = min(run, cap)
            sgs.append((g, take, L))
            g += take
            run -= take
    return sgs


SGS = _supergroups()


def build_bass(reps=1, stages=("gather", "mult", "fold", "store")):
    """Compile the 8-core SPMD program. reps>1 loops the whole pipeline
    on-device (used by test.py for delta timing)."""
    import concourse.bacc as bacc
    import concourse.tile as tile
    from concourse import mybir
    from concourse.library_config import mlp

    nc = bacc.Bacc("TRN2", target_bir_lowering=False, debug=False,
                   num_devices=N_CORES, num_swdge_queues=NQUEUES)
    weight = nc.declare_dram_parameter("weight", [NBLK, EBF],
                                       mybir.dt.bfloat16, isOutput=False)
    idx_in = nc.declare_dram_parameter("idx", [128, ICOLS], mybir.dt.int16,
                                       isOutput=False)
    mask_in = nc.declare_dram_parameter("mask", [128, MCOLS],
                                        mybir.dt.bfloat16, isOutput=False)
    out = nc.declare_dram_parameter("out", [BL, D], mybir.dt.float32,
                                    isOutput=True)
    # row g*128+p -> [p][g][d]
    out_p = out.rearrange("(g p) d -> p g d", p=128)

    qn = 0
    with tile.TileContext(nc) as tc:
        with ExitStack() as ctx:
            cons = ctx.enter_context(tc.tile_pool(name="cons", bufs=1))
            gp = ctx.enter_context(tc.tile_pool(name="g", bufs=4))
            op = ctx.enter_context(tc.tile_pool(name="o", bufs=4))

            nc.gpsimd.load_library(mlp)
            idx_t = cons.tile([128, ICOLS], mybir.dt.int16)
            nc.sync.dma_start(out=idx_t[:], in_=idx_in[:, :])
            mask_t = cons.tile([128, MCOLS], mybir.dt.bfloat16)
            nc.sync.dma_start(out=mask_t[:], in_=mask_in[:, :])

            with tc.For_i(0, reps) as _i:
                # W=1: sequential per-supergroup emission (measured fastest;
                # W=4 instruction interleaving was 20% slower end-to-end).
                W = 1

                def sg_ops(g0, G, L, ic, mc, qbase):
                    T4 = L * 4
                    ni = 128 * L
                    GT4 = G * T4
                    gt = gp.tile([128, G * L * EBF], mybir.dt.bfloat16,
                                 tag="g", name=f"gt{g0}")
                    ot = op.tile([128, G * D], mybir.dt.float32, tag="o",
                                 name=f"ot{g0}")
                    if "gather" in stages:
                        for k in range(G):
                            def do_gather(k=k, gt=gt, ic=ic, ni=ni, L=L,
                                          q=(qbase + k) % NQUEUES):
                                sub = gt[:, k * L * EBF:(k + 1) * L * EBF]
                                nc.gpsimd.dma_gather(
                                    out_ap=sub.rearrange(
                                        "p (c e) -> p c e", e=EBF),
                                    in_ap=weight[:, :],
                                    idxs_ap=idx_t[:, ic + k * ni // 16:
                                                  ic + (k + 1) * ni // 16],
                                    num_idxs=ni,
                                    num_idxs_reg=ni,
                                    elem_size=EBF,
                                    single_packet=False,
                                    queue_num=q,
                                )
                            yield do_gather
                    if "mult" in stages:
                        def do_mult(gt=gt, GT4=GT4, mc=mc):
                            g3 = gt[:].rearrange("p (t d) -> p t d", d=D)
                            m3 = mask_t[:, mc:mc + GT4].to_broadcast(
                                [128, GT4, D])
                            nc.vector.tensor_tensor(
                                out=g3, in0=g3, in1=m3,
                                op=mybir.AluOpType.mult)
                        yield do_mult
                    if "fold" in stages:
                        t = T4
                        while t > 2:
                            h = t // 2
                            def do_fold(gt=gt, G=G, T4=T4, t=t, h=h):
                                g4 = gt[:].rearrange(
                                    "p (g t d) -> p g t d", g=G, t=T4)
                                nc.vector.tensor_tensor(
                                    out=g4[:, :, 0:h, :],
                                    in0=g4[:, :, 0:h, :],
                                    in1=g4[:, :, t - h:t, :],
                                    op=mybir.AluOpType.add)
                            yield do_fold
                            t = t - h

                        def do_final(gt=gt, ot=ot, G=G, T4=T4):
                            g4 = gt[:].rearrange(
                                "p (g t d) -> p g t d", g=G, t=T4)
                            o3 = ot[:].rearrange("p (g d) -> p g d", d=D)
                            nc.vector.tensor_tensor(
                                out=o3, in0=g4[:, :, 0, :],
                                in1=g4[:, :, 1, :],
                                op=mybir.AluOpType.add)
                        yield do_final
                    if "store" in stages:
                        def do_store(ot=ot, g0=g0, G=G):
                            nc.sync.dma_start(
                                out=out_p[:, g0:g0 + G, :],
                                in_=ot[:].rearrange("p (g d) -> p g d", d=D))
                        yield do_store

                gens = []
                ic = 0
                mc = 0
                qn = 0
                for (g0, G, L) in SGS:
                    gens.append(sg_ops(g0, G, L, ic, mc, qn))
                    ic += G * (128 * L) // 16
                    mc += G * L * 4
                    qn += G
                from collections import deque
                window = deque()
                rest = deque(gens)
                while rest or window:
                    while len(window) < W and rest:
                        window.append(rest.popleft())
                    gen = window.popleft()
                    try:
                        step = next(gen)
                    except StopIteration:
                        continue
                    step()
                    window.append(gen)
    nc.compile()
    return nc


def _host_prep(core_items, emb_bag_inputs, offsets, input_lengths):
    """Build per-core idx/mask tensors + the inverse permutation."""
    import ml_dtypes
    it = core_items.astype(np.int64)
    off = offsets[it].astype(np.int64)
    ln = input_lengths[it].astype(np.int64)
    ids = emb_bag_inputs[off[:, None] + np.arange(L_MAX)[None, :]].astype(np.int64)

    order = np.argsort(-ln, kind="stable")      # items sorted by len desc
    ln_s = ln[order]
    ids_s = ids[order]

    idx_arr = np.zeros((128, ICOLS), dtype=np.int16)
    mask_arr = np.zeros((128, MCOLS), dtype=ml_dtypes.bfloat16)
    ic = 0
    mc = 0
    for g, L in enumerate(L_SCHED):
        sl = slice(g * 128, (g + 1) * 128)
        ln_g = ln_s[sl]                          # [128]
        if ln_g.max(initial=0) > L:
            raise RuntimeError(
                f"static lane schedule violated in group {g}: "
                f"max len {ln_g.max()} > {L}")
        ids_g = ids_s[sl]                        # [128, 16]
        lanes = np.minimum(np.arange(L)[None, :], ln_g[:, None] - 1)
        lane_ids = np.take_along_axis(ids_g, lanes, axis=1)  # [128, L]
        blk = (lane_ids >> 2).astype(np.int16)               # [128, L]
        ph = (lane_ids & 3).astype(np.int64)                 # [128, L]
        valid = (np.arange(L)[None, :] < ln_g[:, None])      # [128, L]

        # column-major stream: s = t*128 + p
        stream = blk.T.reshape(-1)                           # [128*L]
        ni = 128 * L
        wrapped = stream.reshape(ni // 16, 16).T             # [16, ni/16]
        idx_arr[:, ic:ic + ni // 16] = np.tile(wrapped, (8, 1))

        m = np.zeros((128, L, 4), dtype=np.float32)
        np.put_along_axis(m, ph[:, :, None], 1.0, axis=2)
        m *= valid[:, :, None]
        mask_arr[:, mc:mc + L * 4] = m.reshape(128, L * 4).astype(
            ml_dtypes.bfloat16)
        ic += ni // 16
        mc += L * 4

    inv = np.empty(BL, dtype=np.int64)
    inv[order] = np.arange(BL)                  # original j -> sorted row
    return idx_arr, mask_arr, inv


def _host_fallback(items, emb_bag_inputs, offsets, input_lengths, weight):
    """Exact numpy fallback (used only if the static schedule is violated,
    i.e. the input distribution differs from the spec)."""
    it = items.astype(np.int64)
    off = offsets[it].astype(np.int64)
    ln = input_lengths[it].astype(np.int64)
    r = np.arange(L_MAX)
    idx = off[:, None] + r[None, :]
    msk = r[None, :] < ln[:, None]
    fid = emb_bag_inputs[np.where(msk, idx, 0)]
    return (weight[fid] * msk[:, :, None].astype(np.float32)).sum(axis=1)


_CACHE = {}


def kernel(items, emb_bag_inputs, offsets, input_lengths, weight):
    import ml_dtypes
    from concourse.bass_utils import run_bass_kernel_spmd

    items = np.asarray(items)
    emb_bag_inputs = np.asarray(emb_bag_inputs)
    offsets = np.asarray(offsets)
    input_lengths = np.asarray(input_lengths)
    weight = np.asarray(weight)

    if "nc" not in _CACHE:
        _CACHE["nc"] = build_bass()
    nc = _CACHE["nc"]

    wblk = np.ascontiguousarray(weight, dtype=np.float32).astype(
        ml_dtypes.bfloat16).reshape(NBLK, EBF)
    in_maps = []
    invs = []
    for c in range(N_CORES):
        try:
            idx_arr, mask_arr, inv = _host_prep(
                items[c * BL:(c + 1) * BL], emb_bag_inputs, offsets,
                input_lengths)
        except RuntimeError:
            return _host_fallback(items, emb_bag_inputs, offsets,
                                  input_lengths,
                                  weight.astype(np.float32)).astype(np.float32)
        in_maps.append({"weight": wblk, "idx": idx_arr, "mask": mask_arr})
        invs.append(inv)

    res = run_bass_kernel_spmd(nc, in_maps, list(range(N_CORES)))
    outs = []
    for c in range(N_CORES):
        dev = res.results[c]["out"]            # [BL, D] in sorted order
        outs.append(dev[invs[c]])
    return np.concatenate(outs, axis=0).astype(np.float32)
